# revision 1
# baseline (speedup 1.0000x reference)
"""Bidirectional chamfer loss kernel for Trainium2 (8 NeuronCores).

Problem (hardcoded): B=2 batches, V1=8192 gt points, V2=8192 pred points, 3D.
  d2[b,i,j] = max(0, |xp_i|^2 + |gt_j|^2 - 2 xp_i.gt_j),  xp = x_pred * mask
  loss_pred2gt[b,i] = sqrt(min_j d2) * 100
  loss_gt2pred[b,j] = sqrt(min_i d2) * 100
  loss_conf = (loss_pred2gt * conf - ln(conf)) * mask ; loss_pred2gt *= mask

Sharding: 8 cores = 2 batches x 4 V2-slices (2048 preds/core vs full 8192 gt).
Each core computes its pred2gt slice exactly, and a partial gt2pred
(min over its 2048 preds); the host combines partials with np.minimum
(sqrt is monotone, so combining after sqrt*100 is exact).

Device kernel (per core, SPMD):
  One K=5 matmul per (pred-tile 128, gt-chunk 512) produces d2 directly in
  PSUM:  A rows [-2 xp_x, -2 xp_y, -2 xp_z, |xp|^2, 1]
         G rows [gt_x,    gt_y,    gt_z,    1,      |gt|^2]
  (host assembles these augmented operands -- pure input layout; all of the
  O(V2*V1) distance/min work runs on device).
  DVE tensor_tensor(min) folds each PSUM tile into a per-pred-tile row
  accumulator and a per-gt-chunk column accumulator; rows finish with a
  free-dim reduce_min, columns with PE 128x128 transposes + reduce_min.

Sync-wait discipline: the TPB ISA allows ONE semaphore wait per
instruction and Tile does not legalize beyond that, so the kernel is
structured so every instruction has at most one cross-engine dependency
not already covered by that engine's earlier waits: accumulator init on
the DVE itself, one fused input DMA per consumer chain, and the transpose
identity laundered through a DVE copy so transposes depend only on the
DVE clock.
"""

import numpy as np

B = 2
V1 = 8192  # gt points
V2 = 8192  # pred points (total)
N_CORES = 8
SLICES = N_CORES // B  # V2-slices per batch
V2C = V2 // SLICES  # pred points per core

_BUILT = {}


def _build(v1, v2c, mm_dtype_name="float32", repeat=1):
    import concourse.tile as tile
    from concourse import bacc, mybir

    f32 = mybir.dt.float32
    mm_dt = getattr(mybir.dt, mm_dtype_name)
    MIN = mybir.AluOpType.min
    MUL = mybir.AluOpType.mult
    SUB = mybir.AluOpType.subtract
    X = mybir.AxisListType.X
    AF = mybir.ActivationFunctionType

    npt = v2c // 128  # pred tiles
    ngc = v1 // 512  # gt chunks (matmul moving dim)
    ngt = v1 // 128  # gt output tiles
    BIG = 3.0e38

    # Bacc (not raw Bass): its compile() legalizes the TRN2 one-wait-per-
    # instruction constraint by splitting sync waits into event semaphores
    nc = bacc.Bacc()
    ag_in = nc.dram_tensor("ag", [5, v2c + v1], mm_dt, kind="ExternalInput")
    mc_in = nc.dram_tensor("mc", [128, 2 * npt], f32, kind="ExternalInput")
    # one fused output -> one DMA queue -> fewer kernel-tail drain waits
    o_all = nc.dram_tensor("o_all", [128, 2 * npt + ngt], f32, kind="ExternalOutput")

    with tile.TileContext(nc) as tc:
        with (
            tc.tile_pool(name="persist", bufs=1) as P,
            tc.tile_pool(name="rowp", bufs=2) as RP,
            tc.tile_pool(name="small", bufs=1) as SP,
            tc.tile_pool(name="mmps", bufs=6, space="PSUM") as MMPS,
            tc.tile_pool(name="trps", bufs=2, space="PSUM") as TRPS,
        ):
            AG = P.tile([5, v2c + v1], mm_dt, tag="AG")
            A = AG[:, 0:v2c]
            G = AG[:, v2c : v2c + v1]
            MC = P.tile([128, 2 * npt], f32, tag="MC")
            mc_sb = P.tile([128, 2 * npt], f32, tag="mc_sb")
            mask_ep = mc_sb[:, 0:npt]
            conf_ep = mc_sb[:, npt : 2 * npt]
            ident_pool = P.tile([128, 128], f32, tag="identp")
            ident = P.tile([128, 128], f32, tag="ident")
            colacc = [
                P.tile([128, 512], f32, tag=f"col{g}", name=f"col{g}")
                for g in range(ngc)
            ]
            p2g_min = P.tile([128, npt], f32, tag="p2gmin")
            g2p_min = P.tile([128, ngt], f32, tag="g2pmin")

            # identity for PE transpose, built on gpsimd then laundered
            # through a DVE copy so its consumers sit in the DVE clock domain
            nc.gpsimd.memset(ident_pool[:], 0.0)
            nc.gpsimd.affine_select(
                out=ident_pool[:],
                in_=ident_pool[:],
                compare_op=mybir.AluOpType.not_equal,
                fill=1.0,
                base=0,
                pattern=[[-1, 128]],
                channel_multiplier=1,
            )
            nc.vector.tensor_copy(ident[:], ident_pool[:])

            # ---- input staging (single DMA per operand) ----
            nc.sync.dma_start(AG[:], ag_in[:, :])
            nc.sync.dma_start(MC[:], mc_in[:, :])
            # pull MC into the DVE's clock domain once; epilogue then has
            # no direct DMA dependencies
            nc.vector.tensor_copy(mc_sb[:], MC[:])

            # accumulator init on the DVE itself (no cross-engine sems)
            for g in range(ngc):
                nc.vector.memset(colacc[g][:], BIG)

            # ---- main loop: one matmul + two DVE min-folds per tile ----
            # repeat>1 re-runs the whole loop (idempotent min-folds) for
            # work-scaling timing experiments
            for pt in [p for _ in range(repeat) for p in range(npt)]:
                rowacc = RP.tile([128, 512], f32, tag="rowacc")
                lhsT = A[:, pt * 128 : (pt + 1) * 128]
                for gc in range(ngc):
                    ps = MMPS.tile([128, 512], f32, tag="mm")
                    nc.tensor.matmul(
                        ps[:],
                        lhsT,
                        G[:, gc * 512 : (gc + 1) * 512],
                        start=True,
                        stop=True,
                    )
                    if gc == 0:
                        nc.vector.tensor_copy(rowacc[:], ps[:])
                    else:
                        nc.vector.tensor_tensor(rowacc[:], rowacc[:], ps[:], op=MIN)
                    nc.vector.tensor_tensor(
                        colacc[gc][:], colacc[gc][:], ps[:], op=MIN
                    )
                nc.vector.tensor_reduce(
                    p2g_min[:, pt : pt + 1], rowacc[:], axis=X, op=MIN
                )

            # ---- column (gt2pred) finish: transpose 128x128 blocks + reduce ----
            for gc in range(ngc):
                for q in range(4):
                    tp = TRPS.tile([128, 128], f32, tag="tr")
                    nc.tensor.transpose(
                        tp[:], colacc[gc][:, q * 128 : (q + 1) * 128], ident[:]
                    )
                    j = gc * 4 + q
                    nc.vector.tensor_reduce(
                        g2p_min[:, j : j + 1], tp[:], axis=X, op=MIN
                    )

            # ---- epilogue ----
            # staged into one SBUF tile [conf | p2g | g2p] whose final
            # producer is always the DVE, so the single output DMA has one wait
            out_sb = SP.tile([128, 2 * npt + ngt], f32, tag="out_sb")
            nc.vector.tensor_scalar_max(p2g_min[:], p2g_min[:], 0.0)
            ep = SP.tile([128, npt], f32, tag="ep")
            # sqrt(10000*x) == 100*sqrt(x)
            nc.scalar.activation(ep[:], p2g_min[:], AF.Sqrt, scale=10000.0)
            lnc = SP.tile([128, npt], f32, tag="lnc")
            nc.scalar.activation(lnc[:], conf_ep[:], AF.Ln)
            nc.vector.tensor_tensor(
                out_sb[:, npt : 2 * npt], ep[:], mask_ep[:], op=MUL
            )
            o2 = SP.tile([128, npt], f32, tag="o2")
            nc.vector.tensor_tensor(o2[:], ep[:], conf_ep[:], op=MUL)
            nc.vector.tensor_tensor(o2[:], o2[:], lnc[:], op=SUB)
            nc.vector.tensor_tensor(out_sb[:, 0:npt], o2[:], mask_ep[:], op=MUL)

            nc.vector.tensor_scalar_max(g2p_min[:], g2p_min[:], 0.0)
            g2 = SP.tile([128, ngt], f32, tag="g2")
            nc.scalar.activation(g2[:], g2p_min[:], AF.Sqrt, scale=10000.0)
            nc.vector.tensor_copy(out_sb[:, 2 * npt :], g2[:])
            nc.sync.dma_start(o_all[:, :], out_sb[:])

    nc.compile()
    return nc


def _build16(v1, v2c, mm_dtype_name="float32", repeat=1, split16=False):
    """fp16 reduction-path variant: the K=5 matmul still runs in
    fp32(+/-r) with exact fp32 PSUM, but each PSUM tile is downconverted
    once by the ScalarE to fp16 in SBUF, so both DVE min-folds run in the
    2x_1P perf mode (2 elem/cycle/lane) instead of fp32-PSUM 1x.
    Cost: one fp16 rounding of d2 (~5e-4 relative) before the min."""
    import concourse.tile as tile
    from concourse import bacc, mybir

    f32 = mybir.dt.float32
    f16 = mybir.dt.float16
    mm_dt = getattr(mybir.dt, mm_dtype_name)
    MIN = mybir.AluOpType.min
    MUL = mybir.AluOpType.mult
    SUB = mybir.AluOpType.subtract
    X = mybir.AxisListType.X
    AF = mybir.ActivationFunctionType

    npt = v2c // 128  # pred tiles
    W = min(2048, v1)  # wide tile: up to 4 matmul chunks, one 4-bank PSUM tile
    ng = v1 // W  # wide gt groups
    nblk = W // 32  # 32-wide blocks per group (DVE transpose)
    BIG16 = 60000.0
    ow = 2 * npt + ng * nblk  # fused output width

    nc = bacc.Bacc()
    S = v2c + v1
    if split16:
        # fp16 hi/lo split operands: d2 = A_hi.G_hi + A_hi.G_lo + A_lo.G_hi
        # (3 fp16 matmuls at 1 cyc/row, PSUM-accumulated; dropped lo.lo
        # term is ~2^-24 relative)
        mm_dt = f16
        ag_in = nc.dram_tensor("ag", [5, 2 * S], f16, kind="ExternalInput")
    else:
        ag_in = nc.dram_tensor("ag", [5, S], mm_dt, kind="ExternalInput")
    mc_in = nc.dram_tensor("mc", [128, 2 * npt], f32, kind="ExternalInput")
    o_all = nc.dram_tensor("o_all", [128, ow], f32, kind="ExternalOutput")

    with tile.TileContext(nc) as tc:
        with (
            tc.tile_pool(name="persist", bufs=1) as P,
            tc.tile_pool(name="rowp", bufs=2) as RP,
            tc.tile_pool(name="s16p", bufs=3) as S16P,
            tc.tile_pool(name="small", bufs=1) as SP,
            tc.tile_pool(name="mmps", bufs=2, space="PSUM") as MMPS,
        ):
            AG = P.tile([5, (2 * S if split16 else S)], mm_dt, tag="AG")
            A = AG[:, 0:v2c]
            G = AG[:, v2c:S]
            A_lo = AG[:, S : S + v2c] if split16 else None
            G_lo = AG[:, S + v2c : 2 * S] if split16 else None
            MC = P.tile([128, 2 * npt], f32, tag="MC")
            mc_sb = P.tile([128, 2 * npt], f32, tag="mc_sb")
            mask_ep = mc_sb[:, 0:npt]
            conf_ep = mc_sb[:, npt : 2 * npt]
            colacc = [
                P.tile([128, W], f16, tag=f"col{g}", name=f"col{g}")
                for g in range(ng)
            ]
            p2g_min = P.tile([128, npt], f32, tag="p2gmin")
            g2p_min = P.tile([32, ng * nblk], f32, tag="g2pmin")

            nc.sync.dma_start(AG[:], ag_in[:, :])
            nc.sync.dma_start(MC[:], mc_in[:, :])
            nc.vector.tensor_copy(mc_sb[:], MC[:])

            for g in range(ng):
                nc.vector.memset(colacc[g][:], BIG16)

            # ---- main loop ----
            for pt in [p for _ in range(repeat) for p in range(npt)]:
                rowacc = RP.tile([128, W], f16, tag="rowacc")
                psl = slice(pt * 128, (pt + 1) * 128)
                lhsT = A[:, psl]
                for g in range(ng):
                    ps = MMPS.tile([128, W], f32, tag="mm")
                    for i in range(W // 512):
                        csl = slice((g * 4 + i) * 512, (g * 4 + i + 1) * 512)
                        if split16:
                            nc.tensor.matmul(
                                ps[:, i * 512 : (i + 1) * 512],
                                lhsT, G[:, csl], start=True, stop=False,
                            )
                            nc.tensor.matmul(
                                ps[:, i * 512 : (i + 1) * 512],
                                lhsT, G_lo[:, csl], start=False, stop=False,
                            )
                            nc.tensor.matmul(
                                ps[:, i * 512 : (i + 1) * 512],
                                A_lo[:, psl], G[:, csl], start=False, stop=True,
                            )
                        else:
                            nc.tensor.matmul(
                                ps[:, i * 512 : (i + 1) * 512],
                                lhsT, G[:, csl], start=True, stop=True,
                            )
                    s16 = S16P.tile([128, W], f16, tag="s16")
                    nc.scalar.copy(s16[:], ps[:])
                    if g == 0:
                        nc.vector.tensor_copy(rowacc[:], s16[:])
                    else:
                        nc.vector.tensor_tensor(rowacc[:], rowacc[:], s16[:], op=MIN)
                    nc.vector.tensor_tensor(
                        colacc[g][:], colacc[g][:], s16[:], op=MIN
                    )
                nc.vector.tensor_reduce(
                    p2g_min[:, pt : pt + 1], rowacc[:], axis=X, op=MIN
                )

            # ---- column (gt2pred) finish, DVE + DMA realign ----
            # 32x32 block transpose + free reduce gives r128[32a+i, b] =
            # min over one partition quarter; DMA realigns quarters to base
            # partition 0 so the final folds have equal base partitions
            # (TT with both SBUF inputs requires equal bases).
            K_ = ng * nblk
            r128 = P.tile([128, K_], f16, tag="r128")
            for g in range(ng):
                tr = SP.tile([128, W], f16, tag="tr", name=f"tr{g}")
                nc.vector.transpose(tr[:], colacc[g][:])
                nc.vector.tensor_reduce(
                    r128[:, g * nblk : (g + 1) * nblk],
                    tr[:].rearrange("p (b j) -> p b j", j=32),
                    axis=X,
                    op=MIN,
                )
            r2 = P.tile([32, 3 * K_], f16, tag="r2")
            for a in range(1, 4):
                nc.sync.dma_start(
                    r2[:, (a - 1) * K_ : a * K_], r128[32 * a : 32 * (a + 1), :]
                )
            g2pm16 = P.tile([32, K_], f16, tag="g2pm16")
            nc.vector.tensor_tensor(g2pm16[:], r128[0:32, :], r2[:, 0:K_], op=MIN)
            nc.vector.tensor_tensor(g2pm16[:], g2pm16[:], r2[:, K_ : 2 * K_], op=MIN)
            nc.vector.tensor_tensor(
                g2pm16[:], g2pm16[:], r2[:, 2 * K_ : 3 * K_], op=MIN
            )
            nc.vector.tensor_copy(g2p_min[:], g2pm16[:])

            # ---- epilogue ----
            out_sb = SP.tile([128, ow], f32, tag="out_sb")
            nc.vector.memset(out_sb[:], 0.0)
            nc.vector.tensor_scalar_max(p2g_min[:], p2g_min[:], 0.0)
            ep = SP.tile([128, npt], f32, tag="ep")
            nc.scalar.activation(ep[:], p2g_min[:], AF.Sqrt, scale=10000.0)
            lnc = SP.tile([128, npt], f32, tag="lnc")
            nc.scalar.activation(lnc[:], conf_ep[:], AF.Ln)
            nc.vector.tensor_tensor(
                out_sb[:, npt : 2 * npt], ep[:], mask_ep[:], op=MUL
            )
            o2 = SP.tile([128, npt], f32, tag="o2")
            nc.vector.tensor_tensor(o2[:], ep[:], conf_ep[:], op=MUL)
            nc.vector.tensor_tensor(o2[:], o2[:], lnc[:], op=SUB)
            nc.vector.tensor_tensor(out_sb[:, 0:npt], o2[:], mask_ep[:], op=MUL)

            nc.vector.tensor_scalar_max(g2p_min[:], g2p_min[:], 0.0)
            g2 = SP.tile([32, ng * nblk], f32, tag="g2")
            nc.scalar.activation(g2[:], g2p_min[:], AF.Sqrt, scale=10000.0)
            nc.vector.tensor_copy(out_sb[0:32, 2 * npt :], g2[:])
            nc.sync.dma_start(o_all[:, :], out_sb[:])

    nc.compile()
    return nc


def get_nc(v1=V1, v2c=V2C, mm_dtype_name="float32", repeat=1, variant="f32"):
    key = (v1, v2c, mm_dtype_name, repeat, variant)
    if key not in _BUILT:
        if variant == "f16x2":
            _BUILT[key] = _build16(v1, v2c, mm_dtype_name, repeat, split16=True)
        elif variant == "f16":
            _BUILT[key] = _build16(v1, v2c, mm_dtype_name, repeat)
        else:
            _BUILT[key] = _build(v1, v2c, mm_dtype_name, repeat)
    return _BUILT[key]


def make_aug(gt, xp):
    """Fused augmented matmul operand [A | G]: one K=5 matmul yields the
    full squared-distance expansion |xp|^2 + |gt|^2 - 2 xp.gt."""
    v2c = xp.shape[0]
    v1 = gt.shape[0]
    ag = np.empty((5, v2c + v1), np.float32)
    ag[0:3, :v2c] = -2.0 * xp.T
    ag[3, :v2c] = (xp * xp).sum(-1)
    ag[4, :v2c] = 1.0
    ag[0:3, v2c:] = gt.T
    ag[3, v2c:] = 1.0
    ag[4, v2c:] = (gt * gt).sum(-1)
    return ag


def make_in_maps(x_gt, x_pred, mask, confidence, split16=False):
    """Shard full inputs into per-core input maps (host-side layout only)."""
    npt = V2C // 128
    in_maps = []
    for c in range(N_CORES):
        b, s = divmod(c, SLICES)
        sl = slice(s * V2C, (s + 1) * V2C)
        xp = x_pred[b, sl] * mask[b, sl, None]  # (V2C, 3) masked preds
        m = mask[b, sl]
        cf = confidence[b, sl]
        ag = make_aug(x_gt[b], xp)
        if split16:
            hi = ag.astype(np.float16)
            lo = (ag - hi.astype(np.float32)).astype(np.float16)
            ag = np.concatenate([hi, lo], axis=1)
        mc = np.empty((128, 2 * npt), np.float32)
        mc[:, :npt] = m.reshape(npt, 128).T
        mc[:, npt:] = cf.reshape(npt, 128).T
        in_maps.append({"ag": ag, "mc": mc})
    return in_maps


def assemble_outputs(results):
    """Gather per-core outputs back to full shapes."""
    loss_conf = np.empty((B, V2), dtype=np.float32)
    loss_p2g = np.empty((B, V2), dtype=np.float32)
    loss_g2p = np.full((B, V1), np.inf, dtype=np.float32)
    for c in range(N_CORES):
        b, s = divmod(c, SLICES)
        sl = slice(s * V2C, (s + 1) * V2C)
        npt = V2C // 128
        o = results[c]["o_all"]
        loss_conf[b, sl] = o[:, 0:npt].T.reshape(V2C)
        loss_p2g[b, sl] = o[:, npt : 2 * npt].T.reshape(V2C)
        if o.shape[1] == 2 * npt + V1 // 128:
            part = o[:, 2 * npt :].T.reshape(V1)  # f32 variant: [p, gtile]
        else:
            # f16 variant: [i, g*64+b] -> gt = g*2048 + 32*b + i
            part = o[0:32, 2 * npt :].T.reshape(V1)
        np.minimum(loss_g2p[b], part, out=loss_g2p[b])
    return loss_conf, loss_p2g, loss_g2p


def kernel(x_gt, x_pred, mask, confidence):
    from concourse.bass_utils import run_bass_kernel_spmd

    nc = get_nc()
    in_maps = make_in_maps(
        np.asarray(x_gt), np.asarray(x_pred), np.asarray(mask), np.asarray(confidence)
    )
    res = run_bass_kernel_spmd(nc, in_maps, list(range(N_CORES)))
    return assemble_outputs(res.results)



# revision 2
# speedup vs baseline: 1.9110x; 1.9110x over previous
"""Bidirectional chamfer loss kernel for Trainium2 (8 NeuronCores).

Problem (hardcoded): B=2 batches, V1=8192 gt points, V2=8192 pred points, 3D.
  d2[b,i,j] = max(0, |xp_i|^2 + |gt_j|^2 - 2 xp_i.gt_j),  xp = x_pred * mask
  loss_pred2gt[b,i] = sqrt(min_j d2) * 100
  loss_gt2pred[b,j] = sqrt(min_i d2) * 100
  loss_conf = (loss_pred2gt * conf - ln(conf)) * mask ; loss_pred2gt *= mask

Sharding: 8 cores = 2 batches x 4 V2-slices (2048 preds/core vs full 8192 gt).
Each core computes its pred2gt slice exactly, and a partial gt2pred
(min over its 2048 preds); the host combines partials with np.minimum
(sqrt is monotone, so combining after sqrt*100 is exact).

Device kernel (per core, SPMD), "k15" variant:
  PE matmul cost is N moving columns regardless of contraction depth K<=128,
  so the fp16 hi/lo split that needs 3 separate matmuls in the naive form
  (A_hi.G_hi + A_lo.G_hi + A_hi.G_lo) is packed into ONE K=15 matmul:
    lhsT rows  0-4  = A_hi   rhs rows  0-4  = G_hi
    lhsT rows  5-9  = A_lo   rhs rows  5-9  = G_hi
    lhsT rows 10-14 = A_hi   rhs rows 10-14 = G_lo
  with A = [-2xp | -2xp_y | -2xp_z | |xp|^2 | 1], G = [gt | 1 | |gt|^2]
  (the K=5 augmented-operand distance expansion). PSUM accumulates in fp32;
  the dropped A_lo.G_lo term is ~2^-22 relative -- fp32-grade d2 at fp16
  matmul cost.

  Per (pred-tile 128, gt-group 2048): 4 N=512 matmuls -> one PSUM tile;
  ScalarE downconverts it once to fp16 SBUF (this enables the DVE 2x_1P
  perf mode); DVE folds it into a per-group column-min accumulator
  (tensor_tensor min) and a per-(tile,group) row min (tensor_reduce).
  Columns finish with DVE 32x32 transposes + reduces as in the f16 path.

Sync-wait discipline: every instruction has at most one cross-engine
dependency (PSUM tile freed by its single ScalarE reader; s16 freed by its
DVE readers; accumulator init on the DVE itself), which Bacc's compile()
legalizes without extra event semaphores.
"""

import numpy as np

B = 2
V1 = 8192  # gt points
V2 = 8192  # pred points (total)
N_CORES = 8
SLICES = N_CORES // B  # V2-slices per batch
V2C = V2 // SLICES  # pred points per core

_BUILT = {}


def _build(v1, v2c, mm_dtype_name="float32", repeat=1):
    import concourse.tile as tile
    from concourse import bacc, mybir

    f32 = mybir.dt.float32
    mm_dt = getattr(mybir.dt, mm_dtype_name)
    MIN = mybir.AluOpType.min
    MUL = mybir.AluOpType.mult
    SUB = mybir.AluOpType.subtract
    X = mybir.AxisListType.X
    AF = mybir.ActivationFunctionType

    npt = v2c // 128  # pred tiles
    ngc = v1 // 512  # gt chunks (matmul moving dim)
    ngt = v1 // 128  # gt output tiles
    BIG = 3.0e38

    nc = bacc.Bacc()
    ag_in = nc.dram_tensor("ag", [5, v2c + v1], mm_dt, kind="ExternalInput")
    mc_in = nc.dram_tensor("mc", [128, 2 * npt], f32, kind="ExternalInput")
    o_all = nc.dram_tensor("o_all", [128, 2 * npt + ngt], f32, kind="ExternalOutput")

    with tile.TileContext(nc) as tc:
        with (
            tc.tile_pool(name="persist", bufs=1) as P,
            tc.tile_pool(name="rowp", bufs=2) as RP,
            tc.tile_pool(name="small", bufs=1) as SP,
            tc.tile_pool(name="mmps", bufs=6, space="PSUM") as MMPS,
            tc.tile_pool(name="trps", bufs=2, space="PSUM") as TRPS,
        ):
            AG = P.tile([5, v2c + v1], mm_dt, tag="AG")
            A = AG[:, 0:v2c]
            G = AG[:, v2c : v2c + v1]
            MC = P.tile([128, 2 * npt], f32, tag="MC")
            mc_sb = P.tile([128, 2 * npt], f32, tag="mc_sb")
            mask_ep = mc_sb[:, 0:npt]
            conf_ep = mc_sb[:, npt : 2 * npt]
            ident_pool = P.tile([128, 128], f32, tag="identp")
            ident = P.tile([128, 128], f32, tag="ident")
            colacc = [
                P.tile([128, 512], f32, tag=f"col{g}", name=f"col{g}")
                for g in range(ngc)
            ]
            p2g_min = P.tile([128, npt], f32, tag="p2gmin")
            g2p_min = P.tile([128, ngt], f32, tag="g2pmin")

            nc.gpsimd.memset(ident_pool[:], 0.0)
            nc.gpsimd.affine_select(
                out=ident_pool[:],
                in_=ident_pool[:],
                compare_op=mybir.AluOpType.not_equal,
                fill=1.0,
                base=0,
                pattern=[[-1, 128]],
                channel_multiplier=1,
            )
            nc.vector.tensor_copy(ident[:], ident_pool[:])

            nc.sync.dma_start(AG[:], ag_in[:, :])
            nc.sync.dma_start(MC[:], mc_in[:, :])
            nc.vector.tensor_copy(mc_sb[:], MC[:])

            for g in range(ngc):
                nc.vector.memset(colacc[g][:], BIG)

            for pt in [p for _ in range(repeat) for p in range(npt)]:
                rowacc = RP.tile([128, 512], f32, tag="rowacc")
                lhsT = A[:, pt * 128 : (pt + 1) * 128]
                for gc in range(ngc):
                    ps = MMPS.tile([128, 512], f32, tag="mm")
                    nc.tensor.matmul(
                        ps[:],
                        lhsT,
                        G[:, gc * 512 : (gc + 1) * 512],
                        start=True,
                        stop=True,
                    )
                    if gc == 0:
                        nc.vector.tensor_copy(rowacc[:], ps[:])
                    else:
                        nc.vector.tensor_tensor(rowacc[:], rowacc[:], ps[:], op=MIN)
                    nc.vector.tensor_tensor(
                        colacc[gc][:], colacc[gc][:], ps[:], op=MIN
                    )
                nc.vector.tensor_reduce(
                    p2g_min[:, pt : pt + 1], rowacc[:], axis=X, op=MIN
                )

            for gc in range(ngc):
                for q in range(4):
                    tp = TRPS.tile([128, 128], f32, tag="tr")
                    nc.tensor.transpose(
                        tp[:], colacc[gc][:, q * 128 : (q + 1) * 128], ident[:]
                    )
                    j = gc * 4 + q
                    nc.vector.tensor_reduce(
                        g2p_min[:, j : j + 1], tp[:], axis=X, op=MIN
                    )

            out_sb = SP.tile([128, 2 * npt + ngt], f32, tag="out_sb")
            nc.vector.tensor_scalar_max(p2g_min[:], p2g_min[:], 0.0)
            ep = SP.tile([128, npt], f32, tag="ep")
            nc.scalar.activation(ep[:], p2g_min[:], AF.Sqrt, scale=10000.0)
            lnc = SP.tile([128, npt], f32, tag="lnc")
            nc.scalar.activation(lnc[:], conf_ep[:], AF.Ln)
            nc.vector.tensor_tensor(
                out_sb[:, npt : 2 * npt], ep[:], mask_ep[:], op=MUL
            )
            o2 = SP.tile([128, npt], f32, tag="o2")
            nc.vector.tensor_tensor(o2[:], ep[:], conf_ep[:], op=MUL)
            nc.vector.tensor_tensor(o2[:], o2[:], lnc[:], op=SUB)
            nc.vector.tensor_tensor(out_sb[:, 0:npt], o2[:], mask_ep[:], op=MUL)

            nc.vector.tensor_scalar_max(g2p_min[:], g2p_min[:], 0.0)
            g2 = SP.tile([128, ngt], f32, tag="g2")
            nc.scalar.activation(g2[:], g2p_min[:], AF.Sqrt, scale=10000.0)
            nc.vector.tensor_copy(out_sb[:, 2 * npt :], g2[:])
            nc.sync.dma_start(o_all[:, :], out_sb[:])

    nc.compile()
    return nc


def _build_k15(v1, v2c, repeat=1, mmw=512):
    """K=15 packed hi/lo fp16 variant (see module docstring)."""
    import concourse.tile as tile
    from concourse import bacc, mybir

    f32 = mybir.dt.float32
    f16 = mybir.dt.float16
    MIN = mybir.AluOpType.min
    MUL = mybir.AluOpType.mult
    SUB = mybir.AluOpType.subtract
    X = mybir.AxisListType.X
    AF = mybir.ActivationFunctionType

    npt = v2c // 128  # pred tiles
    W = min(2048, v1)  # gt group width: one PSUM tile, one ScalarE downconvert
    ng = v1 // W  # gt groups
    nblk = W // 32  # 32-wide blocks per group (DVE transpose)
    K_ = ng * nblk  # total 32-blocks = g2p output columns
    BIG16 = 60000.0
    ow = 2 * npt + K_  # fused output width
    S = v2c + v1

    nc = bacc.Bacc()
    ag_in = nc.dram_tensor("ag", [15, S], f16, kind="ExternalInput")
    mc_in = nc.dram_tensor("mc", [128, 2 * npt], f32, kind="ExternalInput")
    o_all = nc.dram_tensor("o_all", [128, ow], f32, kind="ExternalOutput")

    with tile.TileContext(nc) as tc:
        with (
            tc.tile_pool(name="persist", bufs=1) as P,
            tc.tile_pool(name="s16p", bufs=3) as S16P,
            tc.tile_pool(name="small", bufs=1) as SP,
            tc.tile_pool(name="trp", bufs=2) as TRP,
            tc.tile_pool(name="mmps", bufs=2, space="PSUM") as MMPS,
        ):
            AG = P.tile([15, S], f16, tag="AG")
            A = AG[:, 0:v2c]
            G = AG[:, v2c:S]
            MC = P.tile([128, 2 * npt], f32, tag="MC")
            mc_sb = P.tile([128, 2 * npt], f32, tag="mc_sb")
            mask_ep = mc_sb[:, 0:npt]
            conf_ep = mc_sb[:, npt : 2 * npt]
            colacc = [
                P.tile([128, W], f16, tag=f"col{g}", name=f"col{g}")
                for g in range(ng)
            ]
            rowred = P.tile([128, npt * ng], f16, tag="rowred")
            p2g_min = P.tile([128, npt], f32, tag="p2gmin")
            r128 = P.tile([128, K_], f16, tag="r128")
            r2 = P.tile([32, 3 * K_], f16, tag="r2")
            g2p_min = P.tile([32, K_], f32, tag="g2pmin")

            nc.sync.dma_start(AG[:], ag_in[:, :])
            nc.sync.dma_start(MC[:], mc_in[:, :])
            nc.vector.tensor_copy(mc_sb[:], MC[:])

            for g in range(ng):
                nc.vector.memset(colacc[g][:], BIG16)

            # ---- main loop ----
            for pt in [p for _ in range(repeat) for p in range(npt)]:
                lhsT = A[:, pt * 128 : (pt + 1) * 128]
                for g in range(ng):
                    ps = MMPS.tile([128, W], f32, tag="mm")
                    for i in range(W // mmw):
                        nc.tensor.matmul(
                            ps[:, i * mmw : (i + 1) * mmw],
                            lhsT,
                            G[:, g * W + i * mmw : g * W + (i + 1) * mmw],
                            start=True,
                            stop=True,
                        )
                    s16 = S16P.tile([128, W], f16, tag="s16")
                    nc.scalar.copy(s16[:], ps[:])
                    nc.vector.tensor_tensor(
                        colacc[g][:], colacc[g][:], s16[:], op=MIN
                    )
                    nc.vector.tensor_reduce(
                        rowred[:, pt * ng + g : pt * ng + g + 1],
                        s16[:],
                        axis=X,
                        op=MIN,
                    )

            # fold per-group row mins -> per-tile row min (g innermost)
            nc.vector.tensor_reduce(
                p2g_min[:],
                rowred[:].rearrange("p (t g) -> p t g", g=ng),
                axis=X,
                op=MIN,
            )

            # ---- column (gt2pred) finish: DVE 32x32 transpose + reduce ----
            # r128[32a+i, g*nblk+b] = min over preds in partition-quarter a of
            # colacc[g][:, 32b+i]; DMA realigns quarters to base partition 0
            # (TT with both SBUF inputs requires equal base partitions).
            for g in range(ng):
                tr = TRP.tile([128, W], f16, tag="tr")
                nc.vector.transpose(tr[:], colacc[g][:])
                nc.vector.tensor_reduce(
                    r128[:, g * nblk : (g + 1) * nblk],
                    tr[:].rearrange("p (b j) -> p b j", j=32),
                    axis=X,
                    op=MIN,
                )
            for a in range(1, 4):
                nc.sync.dma_start(
                    r2[:, (a - 1) * K_ : a * K_], r128[32 * a : 32 * (a + 1), :]
                )
            g2pm16 = P.tile([32, K_], f16, tag="g2pm16")
            nc.vector.tensor_tensor(g2pm16[:], r128[0:32, :], r2[:, 0:K_], op=MIN)
            nc.vector.tensor_tensor(g2pm16[:], g2pm16[:], r2[:, K_ : 2 * K_], op=MIN)
            nc.vector.tensor_tensor(
                g2pm16[:], g2pm16[:], r2[:, 2 * K_ : 3 * K_], op=MIN
            )
            nc.vector.tensor_copy(g2p_min[:], g2pm16[:])

            # ---- epilogue ----
            out_sb = SP.tile([128, ow], f32, tag="out_sb")
            nc.vector.memset(out_sb[:], 0.0)
            nc.vector.tensor_scalar_max(p2g_min[:], p2g_min[:], 0.0)
            ep = SP.tile([128, npt], f32, tag="ep")
            # sqrt(10000*x) == 100*sqrt(x)
            nc.scalar.activation(ep[:], p2g_min[:], AF.Sqrt, scale=10000.0)
            lnc = SP.tile([128, npt], f32, tag="lnc")
            nc.scalar.activation(lnc[:], conf_ep[:], AF.Ln)
            nc.vector.tensor_tensor(
                out_sb[:, npt : 2 * npt], ep[:], mask_ep[:], op=MUL
            )
            o2 = SP.tile([128, npt], f32, tag="o2")
            nc.vector.tensor_tensor(o2[:], ep[:], conf_ep[:], op=MUL)
            nc.vector.tensor_tensor(o2[:], o2[:], lnc[:], op=SUB)
            nc.vector.tensor_tensor(out_sb[:, 0:npt], o2[:], mask_ep[:], op=MUL)

            nc.vector.tensor_scalar_max(g2p_min[:], g2p_min[:], 0.0)
            g2 = SP.tile([32, K_], f32, tag="g2")
            nc.scalar.activation(g2[:], g2p_min[:], AF.Sqrt, scale=10000.0)
            nc.vector.tensor_copy(out_sb[0:32, 2 * npt :], g2[:])
            nc.sync.dma_start(o_all[:, :], out_sb[:])

    nc.compile()
    return nc


def get_nc(v1=V1, v2c=V2C, mm_dtype_name="float32", repeat=1, variant="k15"):
    key = (v1, v2c, mm_dtype_name, repeat, variant)
    if key not in _BUILT:
        if variant == "k15":
            _BUILT[key] = _build_k15(v1, v2c, repeat)
        else:
            _BUILT[key] = _build(v1, v2c, mm_dtype_name, repeat)
    return _BUILT[key]


def make_aug(gt, xp):
    """Fused augmented matmul operand [A | G]: one K=5 matmul yields the
    full squared-distance expansion |xp|^2 + |gt|^2 - 2 xp.gt."""
    v2c = xp.shape[0]
    v1 = gt.shape[0]
    ag = np.empty((5, v2c + v1), np.float32)
    ag[0:3, :v2c] = -2.0 * xp.T
    ag[3, :v2c] = (xp * xp).sum(-1)
    ag[4, :v2c] = 1.0
    ag[0:3, v2c:] = gt.T
    ag[3, v2c:] = 1.0
    ag[4, v2c:] = (gt * gt).sum(-1)
    return ag


def make_aug15(gt, xp):
    """K=15 packed hi/lo fp16 operand: rows 0-4 hi.hi, 5-9 A_lo vs G_hi,
    10-14 A_hi vs G_lo (the lo.lo term is dropped, ~2^-22 relative)."""
    v2c = xp.shape[0]
    ag = make_aug(gt, xp)
    hi = ag.astype(np.float16)
    lo = (ag - hi.astype(np.float32)).astype(np.float16)
    ag15 = np.empty((15, ag.shape[1]), np.float16)
    ag15[0:5] = hi
    ag15[5:10, :v2c] = lo[:, :v2c]
    ag15[5:10, v2c:] = hi[:, v2c:]
    ag15[10:15, :v2c] = hi[:, :v2c]
    ag15[10:15, v2c:] = lo[:, v2c:]
    return ag15


def make_in_maps(x_gt, x_pred, mask, confidence, variant="k15"):
    """Shard full inputs into per-core input maps (host-side layout only)."""
    npt = V2C // 128
    in_maps = []
    for c in range(N_CORES):
        b, s = divmod(c, SLICES)
        sl = slice(s * V2C, (s + 1) * V2C)
        xp = x_pred[b, sl] * mask[b, sl, None]  # (V2C, 3) masked preds
        m = mask[b, sl]
        cf = confidence[b, sl]
        if variant == "k15":
            ag = make_aug15(x_gt[b], xp)
        else:
            ag = make_aug(x_gt[b], xp)
        mc = np.empty((128, 2 * npt), np.float32)
        mc[:, :npt] = m.reshape(npt, 128).T
        mc[:, npt:] = cf.reshape(npt, 128).T
        in_maps.append({"ag": ag, "mc": mc})
    return in_maps


def assemble_outputs(results):
    """Gather per-core outputs back to full shapes."""
    loss_conf = np.empty((B, V2), dtype=np.float32)
    loss_p2g = np.empty((B, V2), dtype=np.float32)
    loss_g2p = np.full((B, V1), np.inf, dtype=np.float32)
    for c in range(N_CORES):
        b, s = divmod(c, SLICES)
        sl = slice(s * V2C, (s + 1) * V2C)
        npt = V2C // 128
        o = results[c]["o_all"]
        loss_conf[b, sl] = o[:, 0:npt].T.reshape(V2C)
        loss_p2g[b, sl] = o[:, npt : 2 * npt].T.reshape(V2C)
        if o.shape[1] == 2 * npt + V1 // 128:
            part = o[:, 2 * npt :].T.reshape(V1)  # f32 variant: [p, gtile]
        else:
            # k15 variant: [i, g*nblk+b] -> gt = g*W + 32*b + i
            part = o[0:32, 2 * npt :].T.reshape(V1)
        np.minimum(loss_g2p[b], part, out=loss_g2p[b])
    return loss_conf, loss_p2g, loss_g2p


def kernel(x_gt, x_pred, mask, confidence):
    from concourse.bass_utils import run_bass_kernel_spmd

    nc = get_nc()
    in_maps = make_in_maps(
        np.asarray(x_gt), np.asarray(x_pred), np.asarray(mask), np.asarray(confidence)
    )
    res = run_bass_kernel_spmd(nc, in_maps, list(range(N_CORES)))
    return assemble_outputs(res.results)


# revision 4
# speedup vs baseline: 2.9775x; 1.5580x over previous
"""Bidirectional chamfer loss kernel for Trainium2 (8 NeuronCores).

Problem (hardcoded): B=2 batches, V1=8192 gt points, V2=8192 pred points, 3D.
  d2[b,i,j] = max(0, |xp_i|^2 + |gt_j|^2 - 2 xp_i.gt_j),  xp = x_pred * mask
  loss_pred2gt[b,i] = sqrt(min_j d2) * 100
  loss_gt2pred[b,j] = sqrt(min_i d2) * 100
  loss_conf = (loss_pred2gt * conf - ln(conf)) * mask ; loss_pred2gt *= mask

Sharding: 8 cores = 2 batches x 4 V2-slices (2048 preds/core vs full 8192 gt).
Each core computes its pred2gt slice exactly, and a partial gt2pred
(min over its 2048 preds); the host combines partials with np.minimum
(sqrt is monotone, so combining after sqrt*100 is exact).

Device kernel (per core, SPMD), "k15" variant:
  PE matmul cost is N moving columns regardless of contraction depth K<=128,
  so the fp16 hi/lo split that needs 3 separate matmuls in the naive form
  (A_hi.G_hi + A_lo.G_hi + A_hi.G_lo) is packed into ONE K=15 matmul:
    lhsT rows  0-4  = A_hi   rhs rows  0-4  = G_hi
    lhsT rows  5-9  = A_lo   rhs rows  5-9  = G_hi
    lhsT rows 10-14 = A_hi   rhs rows 10-14 = G_lo
  with A = [-2xp | -2xp_y | -2xp_z | |xp|^2 | 1], G = [gt | 1 | |gt|^2]
  (the K=5 augmented-operand distance expansion). PSUM accumulates in fp32;
  the dropped A_lo.G_lo term is ~2^-22 relative -- fp32-grade d2 at fp16
  matmul cost.

  Per (pred-tile 128, gt-group 2048): 4 N=512 matmuls -> one PSUM tile;
  ScalarE downconverts it once to fp16 SBUF (this enables the DVE 2x_1P
  perf mode); DVE folds it into a per-group column-min accumulator
  (tensor_tensor min) and a per-(tile,group) row min (tensor_reduce).
  Columns finish with DVE 32x32 transposes + reduces as in the f16 path.

Sync-wait discipline: every instruction has at most one cross-engine
dependency (PSUM tile freed by its single ScalarE reader; s16 freed by its
DVE readers; accumulator init on the DVE itself), which Bacc's compile()
legalizes without extra event semaphores.
"""

import numpy as np

B = 2
V1 = 8192  # gt points
V2 = 8192  # pred points (total)
N_CORES = 8
SLICES = N_CORES // B  # V2-slices per batch
V2C = V2 // SLICES  # pred points per core

_BUILT = {}


def _build(v1, v2c, mm_dtype_name="float32", repeat=1):
    import concourse.tile as tile
    from concourse import bacc, mybir

    f32 = mybir.dt.float32
    mm_dt = getattr(mybir.dt, mm_dtype_name)
    MIN = mybir.AluOpType.min
    MUL = mybir.AluOpType.mult
    SUB = mybir.AluOpType.subtract
    X = mybir.AxisListType.X
    AF = mybir.ActivationFunctionType

    npt = v2c // 128  # pred tiles
    ngc = v1 // 512  # gt chunks (matmul moving dim)
    ngt = v1 // 128  # gt output tiles
    BIG = 3.0e38

    nc = bacc.Bacc()
    ag_in = nc.dram_tensor("ag", [5, v2c + v1], mm_dt, kind="ExternalInput")
    mc_in = nc.dram_tensor("mc", [128, 2 * npt], f32, kind="ExternalInput")
    o_all = nc.dram_tensor("o_all", [128, 2 * npt + ngt], f32, kind="ExternalOutput")

    with tile.TileContext(nc) as tc:
        with (
            tc.tile_pool(name="persist", bufs=1) as P,
            tc.tile_pool(name="rowp", bufs=2) as RP,
            tc.tile_pool(name="small", bufs=1) as SP,
            tc.tile_pool(name="mmps", bufs=6, space="PSUM") as MMPS,
            tc.tile_pool(name="trps", bufs=2, space="PSUM") as TRPS,
        ):
            AG = P.tile([5, v2c + v1], mm_dt, tag="AG")
            A = AG[:, 0:v2c]
            G = AG[:, v2c : v2c + v1]
            MC = P.tile([128, 2 * npt], f32, tag="MC")
            mc_sb = P.tile([128, 2 * npt], f32, tag="mc_sb")
            mask_ep = mc_sb[:, 0:npt]
            conf_ep = mc_sb[:, npt : 2 * npt]
            ident_pool = P.tile([128, 128], f32, tag="identp")
            ident = P.tile([128, 128], f32, tag="ident")
            colacc = [
                P.tile([128, 512], f32, tag=f"col{g}", name=f"col{g}")
                for g in range(ngc)
            ]
            p2g_min = P.tile([128, npt], f32, tag="p2gmin")
            g2p_min = P.tile([128, ngt], f32, tag="g2pmin")

            nc.gpsimd.memset(ident_pool[:], 0.0)
            nc.gpsimd.affine_select(
                out=ident_pool[:],
                in_=ident_pool[:],
                compare_op=mybir.AluOpType.not_equal,
                fill=1.0,
                base=0,
                pattern=[[-1, 128]],
                channel_multiplier=1,
            )
            nc.vector.tensor_copy(ident[:], ident_pool[:])

            nc.sync.dma_start(AG[:], ag_in[:, :])
            nc.sync.dma_start(MC[:], mc_in[:, :])
            nc.vector.tensor_copy(mc_sb[:], MC[:])

            for g in range(ngc):
                nc.vector.memset(colacc[g][:], BIG)

            for pt in [p for _ in range(repeat) for p in range(npt)]:
                rowacc = RP.tile([128, 512], f32, tag="rowacc")
                lhsT = A[:, pt * 128 : (pt + 1) * 128]
                for gc in range(ngc):
                    ps = MMPS.tile([128, 512], f32, tag="mm")
                    nc.tensor.matmul(
                        ps[:],
                        lhsT,
                        G[:, gc * 512 : (gc + 1) * 512],
                        start=True,
                        stop=True,
                    )
                    if gc == 0:
                        nc.vector.tensor_copy(rowacc[:], ps[:])
                    else:
                        nc.vector.tensor_tensor(rowacc[:], rowacc[:], ps[:], op=MIN)
                    nc.vector.tensor_tensor(
                        colacc[gc][:], colacc[gc][:], ps[:], op=MIN
                    )
                nc.vector.tensor_reduce(
                    p2g_min[:, pt : pt + 1], rowacc[:], axis=X, op=MIN
                )

            for gc in range(ngc):
                for q in range(4):
                    tp = TRPS.tile([128, 128], f32, tag="tr")
                    nc.tensor.transpose(
                        tp[:], colacc[gc][:, q * 128 : (q + 1) * 128], ident[:]
                    )
                    j = gc * 4 + q
                    nc.vector.tensor_reduce(
                        g2p_min[:, j : j + 1], tp[:], axis=X, op=MIN
                    )

            out_sb = SP.tile([128, 2 * npt + ngt], f32, tag="out_sb")
            nc.vector.tensor_scalar_max(p2g_min[:], p2g_min[:], 0.0)
            ep = SP.tile([128, npt], f32, tag="ep")
            nc.scalar.activation(ep[:], p2g_min[:], AF.Sqrt, scale=10000.0)
            lnc = SP.tile([128, npt], f32, tag="lnc")
            nc.scalar.activation(lnc[:], conf_ep[:], AF.Ln)
            nc.vector.tensor_tensor(
                out_sb[:, npt : 2 * npt], ep[:], mask_ep[:], op=MUL
            )
            o2 = SP.tile([128, npt], f32, tag="o2")
            nc.vector.tensor_tensor(o2[:], ep[:], conf_ep[:], op=MUL)
            nc.vector.tensor_tensor(o2[:], o2[:], lnc[:], op=SUB)
            nc.vector.tensor_tensor(out_sb[:, 0:npt], o2[:], mask_ep[:], op=MUL)

            nc.vector.tensor_scalar_max(g2p_min[:], g2p_min[:], 0.0)
            g2 = SP.tile([128, ngt], f32, tag="g2")
            nc.scalar.activation(g2[:], g2p_min[:], AF.Sqrt, scale=10000.0)
            nc.vector.tensor_copy(out_sb[:, 2 * npt :], g2[:])
            nc.sync.dma_start(o_all[:, :], out_sb[:])

    nc.compile()
    return nc


def _build_k15(v1, v2c, repeat=1, mmw=512):
    """K=15 packed hi/lo fp16 variant (see module docstring)."""
    import concourse.tile as tile
    from concourse import bacc, mybir

    f32 = mybir.dt.float32
    f16 = mybir.dt.float16
    MIN = mybir.AluOpType.min
    MUL = mybir.AluOpType.mult
    SUB = mybir.AluOpType.subtract
    X = mybir.AxisListType.X
    AF = mybir.ActivationFunctionType

    npt = v2c // 128  # pred tiles
    W = min(2048, v1)  # gt group width: one PSUM tile, one ScalarE downconvert
    ng = v1 // W  # gt groups
    nblk = W // 32  # 32-wide blocks per group (DVE transpose)
    K_ = ng * nblk  # total 32-blocks = g2p output columns
    BIG16 = 60000.0
    ow = 2 * npt + K_  # fused output width
    S = v2c + v1

    nc = bacc.Bacc()
    ag_in = nc.dram_tensor("ag", [15, S], f16, kind="ExternalInput")
    mc_in = nc.dram_tensor("mc", [128, 2 * npt], f32, kind="ExternalInput")
    o_all = nc.dram_tensor("o_all", [128, ow], f32, kind="ExternalOutput")

    with tile.TileContext(nc) as tc:
        with (
            tc.tile_pool(name="persist", bufs=1) as P,
            tc.tile_pool(name="s16p", bufs=3) as S16P,
            tc.tile_pool(name="rowp", bufs=2) as RP,
            tc.tile_pool(name="hp", bufs=2) as HP,
            tc.tile_pool(name="small", bufs=1) as SP,
            tc.tile_pool(name="trp", bufs=2) as TRP,
            tc.tile_pool(name="mmps", bufs=2, space="PSUM") as MMPS,
        ):
            AG = P.tile([15, S], f16, tag="AG")
            A = AG[:, 0:v2c]
            G = AG[:, v2c:S]
            MC = P.tile([128, 2 * npt], f32, tag="MC")
            mc_sb = P.tile([128, 2 * npt], f32, tag="mc_sb")
            mask_ep = mc_sb[:, 0:npt]
            conf_ep = mc_sb[:, npt : 2 * npt]
            colacc = [
                P.tile([128, W], f16, tag=f"col{g}", name=f"col{g}")
                for g in range(ng)
            ]
            p2g_min = P.tile([128, npt], f32, tag="p2gmin")
            r128 = P.tile([128, K_], f16, tag="r128")
            r2 = P.tile([32, 3 * K_], f16, tag="r2")
            g2p_min = P.tile([32, K_], f32, tag="g2pmin")

            nc.sync.dma_start(AG[:], ag_in[:, :])
            nc.sync.dma_start(MC[:], mc_in[:, :])
            nc.vector.tensor_copy(mc_sb[:], MC[:])

            # ---- main loop ----
            # Row path avoids the 1x-mode TensorReduce on the hot [128, W]
            # tiles: a TT min of the tile's two halves (2x_1P, both read
            # ports packed -> 4 elem/cycle) + a TT fold into rowacc; only a
            # W/2-wide reduce per pred tile remains at 1x. Col accumulators
            # are seeded by a 4x-mode copy at pt==0 (no memset, no fold).
            H = W // 2
            for pt in [p for _ in range(repeat) for p in range(npt)]:
                lhsT = A[:, pt * 128 : (pt + 1) * 128]
                rowacc = RP.tile([128, H], f16, tag="rowacc")
                for g in range(ng):
                    ps = MMPS.tile([128, W], f32, tag="mm")
                    for i in range(W // mmw):
                        nc.tensor.matmul(
                            ps[:, i * mmw : (i + 1) * mmw],
                            lhsT,
                            G[:, g * W + i * mmw : g * W + (i + 1) * mmw],
                            start=True,
                            stop=True,
                        )
                    s16 = S16P.tile([128, W], f16, tag="s16")
                    nc.scalar.copy(s16[:], ps[:])
                    if pt == 0:
                        nc.vector.tensor_copy(colacc[g][:], s16[:])
                    else:
                        nc.vector.tensor_tensor(
                            colacc[g][:], colacc[g][:], s16[:], op=MIN
                        )
                    if g == 0:
                        nc.vector.tensor_tensor(
                            rowacc[:], s16[:, 0:H], s16[:, H:W], op=MIN
                        )
                    else:
                        h = HP.tile([128, H], f16, tag="h")
                        nc.vector.tensor_tensor(
                            h[:], s16[:, 0:H], s16[:, H:W], op=MIN
                        )
                        nc.vector.tensor_tensor(rowacc[:], rowacc[:], h[:], op=MIN)
                nc.vector.tensor_reduce(
                    p2g_min[:, pt : pt + 1], rowacc[:], axis=X, op=MIN
                )

            # ---- column (gt2pred) finish: DVE 32x32 transpose + reduce ----
            # r128[32a+i, g*nblk+b] = min over preds in partition-quarter a of
            # colacc[g][:, 32b+i]; DMA realigns quarters to base partition 0
            # (TT with both SBUF inputs requires equal base partitions).
            for g in range(ng):
                tr = TRP.tile([128, W], f16, tag="tr")
                nc.vector.transpose(tr[:], colacc[g][:])
                nc.vector.tensor_reduce(
                    r128[:, g * nblk : (g + 1) * nblk],
                    tr[:].rearrange("p (b j) -> p b j", j=32),
                    axis=X,
                    op=MIN,
                )
            for a in range(1, 4):
                nc.sync.dma_start(
                    r2[:, (a - 1) * K_ : a * K_], r128[32 * a : 32 * (a + 1), :]
                )
            g2pm16 = P.tile([32, K_], f16, tag="g2pm16")
            nc.vector.tensor_tensor(g2pm16[:], r128[0:32, :], r2[:, 0:K_], op=MIN)
            nc.vector.tensor_tensor(g2pm16[:], g2pm16[:], r2[:, K_ : 2 * K_], op=MIN)
            nc.vector.tensor_tensor(
                g2pm16[:], g2pm16[:], r2[:, 2 * K_ : 3 * K_], op=MIN
            )
            nc.vector.tensor_copy(g2p_min[:], g2pm16[:])

            # ---- epilogue ----
            out_sb = SP.tile([128, ow], f32, tag="out_sb")
            nc.vector.memset(out_sb[:], 0.0)
            nc.vector.tensor_scalar_max(p2g_min[:], p2g_min[:], 0.0)
            ep = SP.tile([128, npt], f32, tag="ep")
            # sqrt(10000*x) == 100*sqrt(x)
            nc.scalar.activation(ep[:], p2g_min[:], AF.Sqrt, scale=10000.0)
            lnc = SP.tile([128, npt], f32, tag="lnc")
            nc.scalar.activation(lnc[:], conf_ep[:], AF.Ln)
            nc.vector.tensor_tensor(
                out_sb[:, npt : 2 * npt], ep[:], mask_ep[:], op=MUL
            )
            o2 = SP.tile([128, npt], f32, tag="o2")
            nc.vector.tensor_tensor(o2[:], ep[:], conf_ep[:], op=MUL)
            nc.vector.tensor_tensor(o2[:], o2[:], lnc[:], op=SUB)
            nc.vector.tensor_tensor(out_sb[:, 0:npt], o2[:], mask_ep[:], op=MUL)

            nc.vector.tensor_scalar_max(g2p_min[:], g2p_min[:], 0.0)
            g2 = SP.tile([32, K_], f32, tag="g2")
            nc.scalar.activation(g2[:], g2p_min[:], AF.Sqrt, scale=10000.0)
            nc.vector.tensor_copy(out_sb[0:32, 2 * npt :], g2[:])
            nc.sync.dma_start(o_all[:, :], out_sb[:])

    nc.compile()
    return nc


def get_nc(v1=V1, v2c=V2C, mm_dtype_name="float32", repeat=1, variant="k15"):
    key = (v1, v2c, mm_dtype_name, repeat, variant)
    if key not in _BUILT:
        if variant == "k15":
            _BUILT[key] = _build_k15(v1, v2c, repeat)
        else:
            _BUILT[key] = _build(v1, v2c, mm_dtype_name, repeat)
    return _BUILT[key]


def make_aug(gt, xp):
    """Fused augmented matmul operand [A | G]: one K=5 matmul yields the
    full squared-distance expansion |xp|^2 + |gt|^2 - 2 xp.gt."""
    v2c = xp.shape[0]
    v1 = gt.shape[0]
    ag = np.empty((5, v2c + v1), np.float32)
    ag[0:3, :v2c] = -2.0 * xp.T
    ag[3, :v2c] = (xp * xp).sum(-1)
    ag[4, :v2c] = 1.0
    ag[0:3, v2c:] = gt.T
    ag[3, v2c:] = 1.0
    ag[4, v2c:] = (gt * gt).sum(-1)
    return ag


def make_aug15(gt, xp):
    """K=15 packed hi/lo fp16 operand: rows 0-4 hi.hi, 5-9 A_lo vs G_hi,
    10-14 A_hi vs G_lo (the lo.lo term is dropped, ~2^-22 relative)."""
    v2c = xp.shape[0]
    ag = make_aug(gt, xp)
    hi = ag.astype(np.float16)
    lo = (ag - hi.astype(np.float32)).astype(np.float16)
    ag15 = np.empty((15, ag.shape[1]), np.float16)
    ag15[0:5] = hi
    ag15[5:10, :v2c] = lo[:, :v2c]
    ag15[5:10, v2c:] = hi[:, v2c:]
    ag15[10:15, :v2c] = hi[:, :v2c]
    ag15[10:15, v2c:] = lo[:, v2c:]
    return ag15


def make_in_maps(x_gt, x_pred, mask, confidence, variant="k15"):
    """Shard full inputs into per-core input maps (host-side layout only)."""
    npt = V2C // 128
    in_maps = []
    for c in range(N_CORES):
        b, s = divmod(c, SLICES)
        sl = slice(s * V2C, (s + 1) * V2C)
        xp = x_pred[b, sl] * mask[b, sl, None]  # (V2C, 3) masked preds
        m = mask[b, sl]
        cf = confidence[b, sl]
        if variant == "k15":
            ag = make_aug15(x_gt[b], xp)
        else:
            ag = make_aug(x_gt[b], xp)
        mc = np.empty((128, 2 * npt), np.float32)
        mc[:, :npt] = m.reshape(npt, 128).T
        mc[:, npt:] = cf.reshape(npt, 128).T
        in_maps.append({"ag": ag, "mc": mc})
    return in_maps


def assemble_outputs(results):
    """Gather per-core outputs back to full shapes."""
    loss_conf = np.empty((B, V2), dtype=np.float32)
    loss_p2g = np.empty((B, V2), dtype=np.float32)
    loss_g2p = np.full((B, V1), np.inf, dtype=np.float32)
    for c in range(N_CORES):
        b, s = divmod(c, SLICES)
        sl = slice(s * V2C, (s + 1) * V2C)
        npt = V2C // 128
        o = results[c]["o_all"]
        loss_conf[b, sl] = o[:, 0:npt].T.reshape(V2C)
        loss_p2g[b, sl] = o[:, npt : 2 * npt].T.reshape(V2C)
        if o.shape[1] == 2 * npt + V1 // 128:
            part = o[:, 2 * npt :].T.reshape(V1)  # f32 variant: [p, gtile]
        else:
            # k15 variant: [i, g*nblk+b] -> gt = g*W + 32*b + i
            part = o[0:32, 2 * npt :].T.reshape(V1)
        np.minimum(loss_g2p[b], part, out=loss_g2p[b])
    return loss_conf, loss_p2g, loss_g2p


def kernel(x_gt, x_pred, mask, confidence):
    from concourse.bass_utils import run_bass_kernel_spmd

    nc = get_nc()
    in_maps = make_in_maps(
        np.asarray(x_gt), np.asarray(x_pred), np.asarray(mask), np.asarray(confidence)
    )
    res = run_bass_kernel_spmd(nc, in_maps, list(range(N_CORES)))
    return assemble_outputs(res.results)
